# revision 1
# baseline (speedup 1.0000x reference)
"""DotProductAttentionPooling on 8 trn2 NeuronCores.

reference:
    scores = einsum("bld,d->bl", x, q) / sqrt(D)
    scores = where(mask, scores, -inf)
    attn   = nan_to_num(softmax(scores, axis=-1))
    out    = einsum("bl,bld->bd", attn, x)            # [B, D]

Strategy (memory-bound: x is 256 MiB and must be read exactly once):
  - Data-parallel: batch B=32 sharded 4-per-core across 8 cores; query
    replicated; output [B, D] gathered on host.
  - x[b] streams to SBUF in natural layout [128(L-part), chunk, 256(D)]
    with l = p*64 + i so each partition's HBM read is one contiguous
    64 KiB run. Quarter-batch (16-chunk) tiles pipeline DMA / compute.
    The first LOOKAHEAD quarters are issued from three different engine
    queues (sync/gpsimd/tensor) so descriptor pushes don't serialize on
    one sequencer during ramp-up.
  - Per-quarter compute budget is set by DMA pace (~5us/quarter at the
    ~420 GB/s per-core HBM ceiling); every engine stays below it:
      ScalarE: fp32->fp16 convert of the quarter (1 ACTIVATE, ~3.4us)
               + exp of the 16 chunk scores (~0.2us).
      DVE:     one fp16 tensor_tensor product against the pre-scaled,
               16x-replicated query (2x DVE mode, ~2.2us) + 16
               tensor_scalar reduce-accums (4x DVE mode, ~2.0us) that
               produce the per-chunk scores, + the tiny w = exp*mask
               fp16 weight op (~4.4us total).
      TensorE: 16 accumulating [128,1]x[128,256] fp16 pool matmuls
               (~3.3us).
  - Softmax without max-subtraction: scores are O(0.1) so exp cannot
    overflow; the -inf mask becomes w = exp(scores) * mask. The 1/16
    softmax scale is folded into the fp16 query replica. denominator =
    ones-matmul over per-quarter row-sums, +1e-30 so an all-masked
    batch yields 0 (like nan_to_num), not NaN.
  - Pooling: unnormalized acc[1, 256] += w_col.T @ x_chunk as fp16
    accumulating PE matmuls (contraction over partition dim = L);
    final normalize on ScalarE out of PSUM, deferred one batch so it
    never stalls the pipeline.
"""

import numpy as np

B, L, D = 32, 8192, 256
N_CORES = 8
BPC = B // N_CORES        # batches per core
P = 128                   # partitions
CHUNKS = L // P           # 64 L-chunks per batch
QC = 16                   # chunks per quarter tile
NQ = CHUNKS // QC         # quarters per batch
NQT = BPC * NQ            # total quarters per core
LOOKAHEAD = 7             # quarters of DMA prefetch (= xf32 bufs)
N_GPC = 0                 # chunks per quarter converted on GpSimd
N_SC = 0                  # chunks per quarter scored on ScalarE accum
ND = QC - N_SC            # chunks per quarter scored on DVE (fp16 tree)
SCALE = 1.0 / float(np.sqrt(D))

_cache = {}


def _build():
    import concourse.bacc as bacc
    import concourse.bass as bass
    import concourse.tile as tile
    from concourse import mybir

    f32 = mybir.dt.float32
    f16 = mybir.dt.float16
    i32 = mybir.dt.int32
    nc = bacc.Bacc("TRN2", target_bir_lowering=False, debug=False,
                   num_devices=N_CORES)

    x = nc.declare_dram_parameter("x", [BPC, L, D], f32, isOutput=False)
    mask = nc.declare_dram_parameter("mask", [BPC, L], i32, isOutput=False)
    query = nc.declare_dram_parameter("query", [D], f32, isOutput=False)
    out = nc.declare_dram_parameter("out", [BPC, D], f32, isOutput=True)

    # l = p * CHUNKS + i: per-partition HBM reads are contiguous
    x_r = x[:].rearrange("b (p i) d -> b p i d", p=P)
    mask_r = mask[:].rearrange("b (p i) -> b p i", p=P)

    with tile.TileContext(nc) as tc:
        with (
            tc.tile_pool(name="xf32", bufs=LOOKAHEAD) as xf32p,
            tc.tile_pool(name="xf16", bufs=4) as xf16p,
            tc.tile_pool(name="prod", bufs=3) as prodp,
            tc.tile_pool(name="tree", bufs=2) as treep,
            tc.tile_pool(name="small", bufs=4) as small,
            tc.tile_pool(name="scratch", bufs=2) as scratchp,
            tc.tile_pool(name="singles", bufs=1) as singles,
            tc.tile_pool(name="psum", bufs=2, space="PSUM") as psums,
        ):
            # broadcast query across partitions with a rank-1 PE matmul
            # (ones[1,128]^T @ q[1,256]) — a SWDGE broadcast DMA would cost
            # ~15us of head-of-line latency before the first convert
            q_row = singles.tile([1, D], f32)
            nc.sync.dma_start(out=q_row[:],
                              in_=query[:].rearrange("(o d) -> o d", o=1))
            ones_row = singles.tile([1, P], f32)
            nc.vector.memset(ones_row[:], 1.0)
            q_ps = psums.tile([P, D], f32, tag="qbc", name="q_ps")
            nc.tensor.matmul(q_ps[:], ones_row[:], q_row[:],
                             start=True, stop=True)
            # pre-scaled fp16 query, replicated QC times so the per-quarter
            # product is one dense fp16 tensor_tensor (2x DVE mode)
            qs = singles.tile([P, D], f16)
            nc.scalar.activation(out=qs[:], in_=q_ps[:],
                                 func=mybir.ActivationFunctionType.Copy,
                                 scale=SCALE)
            qh_rep = singles.tile([P, QC, D], f16)
            nc.vector.tensor_copy(qh_rep[:, 0, :], qs[:])
            rep = 1
            while rep < QC:
                n = min(rep, QC - rep)
                nc.vector.tensor_copy(qh_rep[:, rep:rep + n, :],
                                      qh_rep[:, 0:n, :])
                rep += n

            xq_tiles = {}       # quarter index -> staged fp32 tile
            xh_tiles = {}       # quarter index -> fp16 copy
            mask_tiles = {}     # batch -> int32 mask tile
            state = {}          # per-batch softmax state
            epilogue = []       # deferred (pool_ps, wsum, b)

            def issue_quarter(k, eng=None):
                eng = eng if eng is not None else nc.sync
                b, qi = divmod(k, NQ)
                xq = xf32p.tile([P, QC, D], f32, tag="xf")
                # first quarters: finer DMA slices so the pipeline fills
                # fast; later ones: a single cheap descriptor push
                ndma = 4 if k == 0 else (2 if k == 1 else 1)
                step = QC // ndma
                for g in range(ndma):
                    sl = slice(qi * QC + g * step, qi * QC + (g + 1) * step)
                    eng.dma_start(out=xq[:, g * step:(g + 1) * step, :],
                                  in_=x_r[b, :, sl, :])
                xq_tiles[k] = xq
                if qi == 0:
                    mi = small.tile([P, CHUNKS], i32, tag="mask_i")
                    nc.sync.dma_start(out=mi[:], in_=mask_r[b])
                    mask_tiles[b] = mi

            def convert_quarter(k, nsplit=1):
                # fp32 -> fp16 cast on ScalarE into a [P, QC, D+1] tile:
                # column D is memset to 1.0 so the pooling matmul's last
                # PSUM column accumulates the softmax denominator for free
                xq = xq_tiles[k]
                xh = xf16p.tile([P, QC, D + 1], f16, tag="xh",
                                name=f"xh{k}")
                nc.gpsimd.memset(xh[:, :, D:D + 1], 1.0)
                step = QC // nsplit
                for g in range(nsplit):
                    nc.scalar.copy(xh[:, g * step:(g + 1) * step, 0:D],
                                   xq[:, g * step:(g + 1) * step, :])
                xh_tiles[k] = xh

            def flush_epilogue():
                while epilogue:
                    pool_ps, bb = epilogue.pop()
                    # +1e-30 so an all-masked batch divides to 0, not NaN
                    den_eps = small.tile([1, 1], f32, tag="den_eps",
                                         name=f"den_eps{bb}")
                    nc.vector.tensor_scalar_add(den_eps[:],
                                                pool_ps[0:1, D:D + 1],
                                                1e-30)
                    rden = small.tile([1, 1], f32, tag="rden",
                                      name=f"rden{bb}")
                    nc.vector.reciprocal(rden[:], den_eps[:])
                    out_sb = small.tile([1, D], f32)
                    nc.scalar.activation(
                        out=out_sb[:], in_=pool_ps[0:1, 0:D],
                        func=mybir.ActivationFunctionType.Copy,
                        scale=rden[0:1, 0:1])
                    nc.sync.dma_start(out=out[bb:bb + 1, :], in_=out_sb[:])

            # all x DMAs go on the single sync HWDGE ring in quarter order:
            # a second ring (scalar) interleaves at the DMA engines and
            # delays quarter 0's arrival, which gates the first compute
            for k in range(min(LOOKAHEAD, NQT)):
                issue_quarter(k)

            for k in range(NQT):
                b, qi = divmod(k, NQ)
                if qi == 0:
                    state[b] = {
                        "pool_ps": psums.tile([1, D + 1], f32, tag="pool",
                                              name=f"pool_ps{b}", bufs=3),
                        "mask_f": small.tile([P, CHUNKS], f32, tag="mask_f",
                                             name=f"mask_f{b}"),
                    }
                st = state[b]

                # convert quarter k+1 ahead of its product, split in two
                # halves so the scheduler can interleave the per-quarter
                # exp between them on ScalarE without serializing
                if k == 0:
                    convert_quarter(0, nsplit=4)
                xq = xq_tiles.pop(k)
                xh = xh_tiles.pop(k)
                if k + 1 < NQT:
                    convert_quarter(k + 1, nsplit=2)

                if qi == 0:
                    # mask -> additive bias: 0 where kept, -40 where masked,
                    # so the single Exp below yields w = exp(score)*mask
                    # (exp(-40+s) flushes to 0 in fp16)
                    nc.vector.tensor_copy(st["mask_f"][:],
                                          mask_tiles[b][:])
                    nc.vector.tensor_scalar(
                        out=st["mask_f"][:], in0=st["mask_f"][:],
                        scalar1=40.0, scalar2=40.0,
                        op0=mybir.AluOpType.mult,
                        op1=mybir.AluOpType.subtract)

                scores_q = small.tile([P, QC], f32, tag="scores")

                # dense fp16 product against the replicated pre-scaled
                # query (2x DVE mode) for all 16 chunks
                prodh = prodp.tile([P, QC, D], f16, tag="prodh")
                nc.vector.tensor_tensor(out=prodh[:],
                                        in0=xh[:, :, 0:D],
                                        in1=qh_rep[:],
                                        op=mybir.AluOpType.mult)

                # per-chunk reduce on DVE, all in 2x fp16 mode: a
                # pairwise-add tree 256->16, then one small 1x reduce
                t1 = treep.tile([P, ND, 128], f16, tag="t1")
                nc.vector.tensor_tensor(out=t1[:],
                                        in0=prodh[:, 0:ND, 0:128],
                                        in1=prodh[:, 0:ND, 128:256],
                                        op=mybir.AluOpType.add)
                t2 = treep.tile([P, ND, 64], f16, tag="t2")
                nc.vector.tensor_tensor(out=t2[:], in0=t1[:, :, 0:64],
                                        in1=t1[:, :, 64:128],
                                        op=mybir.AluOpType.add)
                t3 = treep.tile([P, ND, 32], f16, tag="t3")
                nc.vector.tensor_tensor(out=t3[:], in0=t2[:, :, 0:32],
                                        in1=t2[:, :, 32:64],
                                        op=mybir.AluOpType.add)
                nc.vector.tensor_reduce(out=scores_q[:, 0:ND], in_=t3[:],
                                        op=mybir.AluOpType.add,
                                        axis=mybir.AxisListType.X)

                if b > 0 and qi == 0:
                    flush_epilogue()
                if k + LOOKAHEAD < NQT:
                    issue_quarter(k + LOOKAHEAD)

                # per-quarter softmax tail: add the mask bias, then ONE
                # Exp with accum produces both the fp16 weights and the
                # quarter's denominator column
                sl = slice(qi * QC, (qi + 1) * QC)
                scores_m = small.tile([P, QC], f32, tag="scores_m")
                nc.vector.tensor_tensor(out=scores_m[:], in0=scores_q[:],
                                        in1=st["mask_f"][:, sl],
                                        op=mybir.AluOpType.add)
                wqh = small.tile([P, QC], f16, tag="wqh")
                nc.scalar.activation(out=wqh[:], in_=scores_m[:],
                                     func=mybir.ActivationFunctionType.Exp)

                for i in range(QC):
                    nc.tensor.matmul(
                        st["pool_ps"][:],
                        wqh[:, i:i + 1],
                        xh[:, i, :],
                        start=(qi == 0 and i == 0),
                        stop=(qi == NQ - 1 and i == QC - 1),
                    )

                if qi == NQ - 1:
                    epilogue.append((st["pool_ps"], b))
                    del state[b]

            flush_epilogue()

    nc.compile()
    return nc


def kernel(x: np.ndarray, mask: np.ndarray, query: np.ndarray) -> np.ndarray:
    from concourse.bass_utils import run_bass_kernel_spmd

    if "nc" not in _cache:
        _cache["nc"] = _build()
    nc = _cache["nc"]

    x = np.ascontiguousarray(np.asarray(x, dtype=np.float32))
    mask = np.ascontiguousarray(np.asarray(mask, dtype=np.int32))
    query = np.ascontiguousarray(np.asarray(query, dtype=np.float32))

    in_maps = [
        {
            "x": np.ascontiguousarray(x[c * BPC:(c + 1) * BPC]),
            "mask": np.ascontiguousarray(mask[c * BPC:(c + 1) * BPC]),
            "query": query,
        }
        for c in range(N_CORES)
    ]
    res = run_bass_kernel_spmd(nc, in_maps, core_ids=list(range(N_CORES)))
    return np.concatenate([res.results[c]["out"] for c in range(N_CORES)], axis=0)



# revision 6
# speedup vs baseline: 1.0701x; 1.0701x over previous
"""DotProductAttentionPooling on 8 trn2 NeuronCores.

reference:
    scores = einsum("bld,d->bl", x, q) / sqrt(D)
    scores = where(mask, scores, -inf)
    attn   = nan_to_num(softmax(scores, axis=-1))
    out    = einsum("bl,bld->bd", attn, x)            # [B, D]

Strategy (memory-bound: x is 256 MiB and must be read exactly once):
  - Data-parallel: batch B=32 sharded 4-per-core across 8 cores; query
    replicated; output [B, D] gathered on host.
  - x[b] streams to SBUF with l = p*64 + i so each partition's HBM read
    is one contiguous run. The DMA itself casts fp32 -> fp16 (SWDGE
    inline convert), so no engine spends time on dtype conversion and
    SBUF holds only the fp16 copy (deep prefetch pipeline).
  - Scores on DVE: one dense fp16 tensor_tensor product against the
    pre-scaled 16x-replicated query (2x DVE mode), then a pairwise-add
    tree 256->32 and a final small tensor_reduce. (tensor_tensor_reduce
    would fuse this but its ucode faults the exec unit on this HW.)
  - Softmax without max-subtraction: scores are O(0.1) so exp cannot
    overflow; the -inf mask becomes a -40 additive bias (exp -> 0 in
    fp16). The 1/16 softmax scale is folded into the fp16 query.
  - Pooling: unnormalized acc[1, 256] += w_col.T @ x_chunk as fp16
    accumulating PE matmuls (contraction over partition dim = L).
    Denominator: one extra tiny matmul per quarter accumulates
    ones.T @ wqh into a [1, 16] PSUM tile. Final normalize on ScalarE
    out of PSUM, deferred one batch so it never stalls the pipeline.
"""

import numpy as np

B, L, D = 32, 8192, 256
N_CORES = 8
BPC = B // N_CORES        # batches per core
P = 128                   # partitions
CHUNKS = L // P           # 64 L-chunks per batch
QC = 16                   # chunks per quarter tile
NQ = CHUNKS // QC         # quarters per batch
NQT = BPC * NQ            # total quarters per core
LOOKAHEAD = 10            # quarters of fp16 DMA prefetch
SCALE = 1.0 / float(np.sqrt(D))

_cache = {}


def _build():
    import concourse.bacc as bacc
    import concourse.bass as bass
    import concourse.tile as tile
    from concourse import mybir

    f32 = mybir.dt.float32
    f16 = mybir.dt.float16
    i32 = mybir.dt.int32
    nc = bacc.Bacc("TRN2", target_bir_lowering=False, debug=False,
                   num_devices=N_CORES)

    x = nc.declare_dram_parameter("x", [BPC, L, D], f32, isOutput=False)
    mask = nc.declare_dram_parameter("mask", [BPC, L], i32, isOutput=False)
    query = nc.declare_dram_parameter("query", [D], f32, isOutput=False)
    out = nc.declare_dram_parameter("out", [BPC, D], f32, isOutput=True)

    # l = p * CHUNKS + i: per-partition HBM reads are contiguous
    x_r = x[:].rearrange("b (p i) d -> b p i d", p=P)
    mask_r = mask[:].rearrange("b (p i) -> b p i", p=P)

    with tile.TileContext(nc) as tc:
        with (
            tc.tile_pool(name="xh", bufs=LOOKAHEAD) as xhp,
            tc.tile_pool(name="prod", bufs=2) as prodp,
            tc.tile_pool(name="tree", bufs=2) as treep,
            tc.tile_pool(name="small", bufs=4) as small,
            tc.tile_pool(name="singles", bufs=1) as singles,
            tc.tile_pool(name="psum", bufs=2, space="PSUM") as psums,
        ):
            # broadcast query across partitions with a rank-1 PE matmul
            # (ones[1,128]^T @ q[1,256]) — a SWDGE broadcast DMA would cost
            # ~15us of head-of-line latency before the first compute
            q_row = singles.tile([1, D], f32)
            nc.sync.dma_start(out=q_row[:],
                              in_=query[:].rearrange("(o d) -> o d", o=1))
            ones_row = singles.tile([1, P], f32)
            nc.vector.memset(ones_row[:], 1.0)
            ones_col = singles.tile([P, 1], f16)
            nc.vector.memset(ones_col[:], 1.0)
            q_ps = psums.tile([P, D], f32, tag="qbc", name="q_ps")
            nc.tensor.matmul(q_ps[:], ones_row[:], q_row[:],
                             start=True, stop=True)
            # pre-scaled fp16 query (softmax 1/sqrt(D) folded in),
            # replicated QC times so the per-quarter product is one dense
            # fp16 tensor_tensor (2x DVE mode)
            qs = singles.tile([P, D], f16)
            nc.scalar.activation(out=qs[:], in_=q_ps[:],
                                 func=mybir.ActivationFunctionType.Copy,
                                 scale=SCALE)
            qh_rep = singles.tile([P, QC, D], f16)
            nc.vector.tensor_copy(qh_rep[:, 0, :], qs[:])
            rep = 1
            while rep < QC:
                n = min(rep, QC - rep)
                nc.vector.tensor_copy(qh_rep[:, rep:rep + n, :],
                                      qh_rep[:, 0:n, :])
                rep += n

            xh_tiles = {}       # quarter index -> staged fp16 tile
            mask_tiles = {}     # batch -> int32 mask tile
            state = {}          # per-batch softmax state
            epilogue = []       # deferred (pool_ps, den_ps, b)

            def issue_quarter(k):
                b, qi = divmod(k, NQ)
                xh = xhp.tile([P, QC, D], f16, tag="xh")
                # first quarters: finer DMA slices so the pipeline fills
                # fast; later ones: a single cheap descriptor push
                ndma = 4 if k == 0 else (2 if k == 1 else 1)
                step = QC // ndma
                for g in range(ndma):
                    sl = slice(qi * QC + g * step, qi * QC + (g + 1) * step)
                    nc.gpsimd.dma_start(out=xh[:, g * step:(g + 1) * step, :],
                                        in_=x_r[b, :, sl, :])
                xh_tiles[k] = xh
                if qi == 0:
                    mi = small.tile([P, CHUNKS], i32, tag="mask_i")
                    nc.sync.dma_start(out=mi[:], in_=mask_r[b])
                    mask_tiles[b] = mi

            def flush_epilogue():
                while epilogue:
                    pool_ps, den_ps, bb = epilogue.pop()
                    den_sum = small.tile([1, 1], f32, tag="den_sum",
                                         name=f"den_sum{bb}")
                    nc.vector.tensor_reduce(out=den_sum[:], in_=den_ps[:],
                                            op=mybir.AluOpType.add,
                                            axis=mybir.AxisListType.X)
                    # +1e-30 so an all-masked batch divides to 0, not NaN
                    den_eps = small.tile([1, 1], f32, tag="den_eps",
                                         name=f"den_eps{bb}")
                    nc.vector.tensor_scalar_add(den_eps[:], den_sum[:],
                                                1e-30)
                    rden = small.tile([1, 1], f32, tag="rden",
                                      name=f"rden{bb}")
                    nc.vector.reciprocal(rden[:], den_eps[:])
                    out_sb = small.tile([1, D], f32)
                    nc.scalar.activation(
                        out=out_sb[:], in_=pool_ps[0:1, 0:D],
                        func=mybir.ActivationFunctionType.Copy,
                        scale=rden[0:1, 0:1])
                    nc.sync.dma_start(out=out[bb:bb + 1, :], in_=out_sb[:])

            for k in range(min(LOOKAHEAD, NQT)):
                issue_quarter(k)

            for k in range(NQT):
                b, qi = divmod(k, NQ)
                if qi == 0:
                    state[b] = {
                        "pool_ps": psums.tile([1, D], f32, tag="pool",
                                              name=f"pool_ps{b}", bufs=3),
                        "den_ps": psums.tile([1, QC], f32, tag="den",
                                             name=f"den_ps{b}", bufs=3),
                        "mask_f": small.tile([P, CHUNKS], f32, tag="mask_f",
                                             name=f"mask_f{b}"),
                    }
                st = state[b]

                xh = xh_tiles.pop(k)

                if qi == 0:
                    # mask -> additive bias: 0 where kept, -40 where masked,
                    # folded into the score reduction's initial value so the
                    # single Exp below yields w = exp(score)*mask
                    nc.vector.tensor_copy(st["mask_f"][:],
                                          mask_tiles[b][:])
                    nc.vector.tensor_scalar(
                        out=st["mask_f"][:], in0=st["mask_f"][:],
                        scalar1=40.0, scalar2=40.0,
                        op0=mybir.AluOpType.mult,
                        op1=mybir.AluOpType.subtract)

                if b > 0 and qi == 0:
                    flush_epilogue()
                if k + LOOKAHEAD < NQT:
                    issue_quarter(k + LOOKAHEAD)

                # dense fp16 product against the replicated pre-scaled
                # query (2x DVE mode) for all 16 chunks
                scores_q = small.tile([P, QC], f32, tag="scores")
                prodh = prodp.tile([P, QC, D], f16, tag="prodh")
                nc.vector.tensor_tensor(out=prodh[:],
                                        in0=xh[:],
                                        in1=qh_rep[:],
                                        op=mybir.AluOpType.mult)

                # per-chunk reduce on DVE, all in 2x fp16 mode: a
                # pairwise-add tree 256->32, then one small 1x reduce
                t1 = treep.tile([P, QC, 128], f16, tag="t1")
                nc.vector.tensor_tensor(out=t1[:],
                                        in0=prodh[:, :, 0:128],
                                        in1=prodh[:, :, 128:256],
                                        op=mybir.AluOpType.add)
                t2 = treep.tile([P, QC, 64], f16, tag="t2")
                nc.vector.tensor_tensor(out=t2[:], in0=t1[:, :, 0:64],
                                        in1=t1[:, :, 64:128],
                                        op=mybir.AluOpType.add)
                t3 = treep.tile([P, QC, 32], f16, tag="t3")
                nc.vector.tensor_tensor(out=t3[:], in0=t2[:, :, 0:32],
                                        in1=t2[:, :, 32:64],
                                        op=mybir.AluOpType.add)
                nc.vector.tensor_reduce(out=scores_q[:], in_=t3[:],
                                        op=mybir.AluOpType.add,
                                        axis=mybir.AxisListType.X)

                # add the mask bias, then ONE Exp produces the fp16
                # weights for this quarter
                sl = slice(qi * QC, (qi + 1) * QC)
                scores_m = small.tile([P, QC], f32, tag="scores_m")
                nc.vector.tensor_tensor(out=scores_m[:], in0=scores_q[:],
                                        in1=st["mask_f"][:, sl],
                                        op=mybir.AluOpType.add)
                wqh = small.tile([P, QC], f16, tag="wqh")
                nc.scalar.activation(out=wqh[:], in_=scores_m[:],
                                     func=mybir.ActivationFunctionType.Exp)

                # denominator: ones^T @ wqh accumulates per-chunk weight
                # sums into a [1, QC] PSUM tile
                nc.tensor.matmul(st["den_ps"][:], ones_col[:], wqh[:],
                                 start=(qi == 0), stop=(qi == NQ - 1))

                for i in range(QC):
                    nc.tensor.matmul(
                        st["pool_ps"][:],
                        wqh[:, i:i + 1],
                        xh[:, i, :],
                        start=(qi == 0 and i == 0),
                        stop=(qi == NQ - 1 and i == QC - 1),
                    )

                if qi == NQ - 1:
                    epilogue.append((st["pool_ps"], st["den_ps"], b))
                    del state[b]

            flush_epilogue()

    nc.compile()
    return nc


def kernel(x: np.ndarray, mask: np.ndarray, query: np.ndarray) -> np.ndarray:
    from concourse.bass_utils import run_bass_kernel_spmd

    if "nc" not in _cache:
        _cache["nc"] = _build()
    nc = _cache["nc"]

    x = np.ascontiguousarray(np.asarray(x, dtype=np.float32))
    mask = np.ascontiguousarray(np.asarray(mask, dtype=np.int32))
    query = np.ascontiguousarray(np.asarray(query, dtype=np.float32))

    in_maps = [
        {
            "x": np.ascontiguousarray(x[c * BPC:(c + 1) * BPC]),
            "mask": np.ascontiguousarray(mask[c * BPC:(c + 1) * BPC]),
            "query": query,
        }
        for c in range(N_CORES)
    ]
    res = run_bass_kernel_spmd(nc, in_maps, core_ids=list(range(N_CORES)))
    return np.concatenate([res.results[c]["out"] for c in range(N_CORES)], axis=0)
